# revision 5
# baseline (speedup 1.0000x reference)
"""Trainium2 Bass kernel for nn_ArrivalTime (sparse attention over 24 timeslots).

Math refactoring (exact, up to fp reassociation):
  query = [user_pref[user], timeslot[hour]] has only 64 distinct user rows and
  24 distinct time rows, so
    scores[n,h,t] = US[b(n), h, t] + TS[hour[n], h, t]
  with tiny tables
    US = (user_q @ k^T) * scale   [64, H*T]   (user_q folds bq)
    TS = (time_q @ k^T) * scale   [24, H*T]
  Masking adds -1e9 where hour_mask==1.  Softmax per head over t (24).
  Output: out[n,:] = attn[n, :] @ vproj + bu, vproj[(h,t),d] = v[h,t,:]@Wu[d,h*HD:]^T.

Device layout (per core, transposed: tokens on the free dim):
  scores_T [96, 512] = table[48, 97]^T @ stream[48, 512]  (one-hot hour + mask rows)
  p = exp(scores_T + US_b bias)   (row 96 = exp(0) = 1 -> folds bu via vproj row 96)
  Z = seg^T @ p ; r = 1/Z ; r_rep = segT^T @ r ; attn = p * r_rep  (in place)
  out_T[d,n] = vproj_ext[:,d]^T @ p  (two 128-col halves, bu folded)

Sharding: data-parallel over batch, 8 batch rows (= 8 x 512 tokens) per core.
"""

import os
import numpy as np

B, S, D, H, HD, T = 64, 512, 256, 4, 64, 24
NCORES = 8
BPC = B // NCORES  # batch rows per core
HT = H * T  # 96
K1 = 2 * T  # 48 rows: one-hot hour + mask
MASK_NEG = -1.0e9

# packed constant tensor [HT+1, CW]: columns
#   0:BPC               us bias columns
#   BPC:BPC+D           vproj_ext
#   BPC+D:BPC+D+H       seg
#   BPC+D+H : +HT+1     table   (valid rows :K1)
#   ... : +HT           segT    (valid rows :H)
C_US = 0
C_VP = C_US + BPC
C_SEG = C_VP + D
C_TAB = C_SEG + H
C_SEGT = C_TAB + (HT + 1)
CW = C_SEGT + HT


def _host_tables(timeslot_embedded, user, hour, hour_mask, user_pref,
                 Wq, bq, Wk, bk, Wv, bv, Wu, bu):
    f32 = np.float32
    ts_e = np.asarray(timeslot_embedded, f32)          # [T, D]
    user = np.asarray(user).astype(np.int64)           # [B]
    hour = np.asarray(hour).astype(np.int64)           # [B, S]
    hour_mask = np.asarray(hour_mask)                  # [B, S, T]
    Wq = np.asarray(Wq, f32); bq = np.asarray(bq, f32)
    Wk = np.asarray(Wk, f32); bk = np.asarray(bk, f32)
    Wv = np.asarray(Wv, f32); bv = np.asarray(bv, f32)
    Wu = np.asarray(Wu, f32); bu = np.asarray(bu, f32)

    Wq_u, Wq_t = Wq[:, :, :D], Wq[:, :, D:]
    k_ = np.einsum('td,hkd->htk', ts_e, Wk) + bk[:, None, :]   # [H,T,HD]
    v_ = np.einsum('td,hkd->htk', ts_e, Wv) + bv[:, None, :]
    time_q = np.einsum('td,hkd->thk', ts_e, Wq_t)              # [T,H,HD]
    upref = np.asarray(user_pref, f32)[user]                   # [B,D]
    user_q = np.einsum('bd,hkd->bhk', upref, Wq_u) + bq[None]  # [B,H,HD]
    scale = f32(1.0 / np.sqrt(HD))
    TS = (np.einsum('thk,hsk->ths', time_q, k_) * scale).reshape(T, HT)
    US = (np.einsum('bhk,hsk->bhs', user_q, k_) * scale).reshape(B, HT)
    vproj = np.einsum('htk,dhk->htd', v_, Wu.reshape(D, H, HD)).reshape(HT, D)

    # table [K1, HT+1]: rows 0..23 TS, rows 24..47 mask additive; col 96 = 0
    table = np.zeros((K1, HT + 1), f32)
    table[:T, :HT] = TS
    table[T:, :HT] = np.tile(np.eye(T, dtype=f32), (1, H)) * f32(MASK_NEG)

    seg = np.zeros((HT + 1, H), f32)
    seg[:HT] = np.repeat(np.eye(H, dtype=f32), T, axis=0)
    segT = np.ascontiguousarray(seg[:HT].T)                    # [H, HT]

    consts = []
    for c in range(NCORES):
        cc = np.zeros((HT + 1, CW), f32)
        cc[:HT, C_US:C_US + BPC] = US[c * BPC:(c + 1) * BPC].T
        cc[:HT, C_VP:C_VP + D] = vproj
        cc[HT, C_VP:C_VP + D] = bu
        cc[:, C_SEG:C_SEG + H] = seg
        cc[:K1, C_TAB:C_TAB + HT + 1] = table
        cc[:H, C_SEGT:C_SEGT + HT] = segT
        consts.append(cc)

    # per-core streams [BPC, K1, S]: one-hot(hour) rows + mask^T rows
    eyeT = np.eye(T, dtype=f32)
    streams = []
    for c in range(NCORES):
        hb = hour[c * BPC:(c + 1) * BPC]                       # [BPC, S]
        mb = hour_mask[c * BPC:(c + 1) * BPC]                  # [BPC, S, T]
        st = np.empty((BPC, K1, S), f32)
        st[:, :T, :] = eyeT[hb].transpose(0, 2, 1)
        st[:, T:, :] = mb.astype(f32).transpose(0, 2, 1)
        streams.append(st)
    return consts, streams


def _build_program():
    import concourse.bass as bass
    import concourse.mybir as mybir

    f32 = mybir.dt.float32
    nc = bass.Bass("TRN2")
    stream_d = nc.declare_dram_parameter("stream", [BPC, K1, S], f32, isOutput=False)
    const_d = nc.declare_dram_parameter("const", [HT + 1, CW], f32, isOutput=False)
    out_d = nc.declare_dram_parameter("out", [BPC, D, S], f32, isOutput=True)

    Exp = mybir.ActivationFunctionType.Exp

    # Raw bass: walrus in this toolchain allows at most ONE attached sem wait
    # per engine instruction, so all waits are standalone wait_ge ops.
    # Engine ticks per iteration i:
    #   PE : A=5i+1  headsum=5i+2  bcast=5i+3  mm2a=5i+4  mm2b=5i+5
    #   ACT: exp=2i+1  ocopy0=2i+2   (+ out-DMA triggers, no tick)
    #   DVE: recip=3i+1  mul=3i+2  ocopy1=3i+3
    from contextlib import ExitStack
    with ExitStack() as ctx:
        ec = ctx.enter_context
        const_sb = ec(nc.sbuf_tensor("const_sb", [HT + 1, CW], f32))
        sts = [ec(nc.sbuf_tensor(f"st{j}", [K1, S], f32)) for j in range(3)]
        ps = [ec(nc.sbuf_tensor(f"p{j}", [HT + 1, S], f32)) for j in range(2)]
        r_sb = ec(nc.sbuf_tensor("r_sb", [H, S], f32))
        ots = [[ec(nc.sbuf_tensor(f"ot{h}{j}", [128, S], f32))
                for j in range(2)] for h in range(2)]
        ps_ss = [ec(nc.psum_tensor(f"ps_s{j}", [HT + 1, S], f32))
                 for j in range(2)]
        ps_z = ec(nc.psum_tensor("ps_z", [H, S], f32))
        ps_r = ec(nc.psum_tensor("ps_r", [HT, S], f32))
        ps_os = [[ec(nc.psum_tensor(f"ps_o{h}{j}", [128, S], f32))
                  for j in range(2)] for h in range(2)]
        c_sem = ec(nc.semaphore("c_sem"))
        st_sems = [ec(nc.semaphore(f"st_sem{j}")) for j in range(3)]
        pe_sem = ec(nc.semaphore("pe_sem"))
        act_sem = ec(nc.semaphore("act_sem"))
        dve_sem = ec(nc.semaphore("dve_sem"))
        ot_sems = [[ec(nc.semaphore(f"ot_sem{h}{j}")) for j in range(2)]
                   for h in range(2)]
        block = ec(nc.Block())

        us = const_sb[:, C_US:C_US + BPC]
        vproj = const_sb[:, C_VP:C_VP + D]
        seg = const_sb[:, C_SEG:C_SEG + H]
        table = const_sb[:K1, C_TAB:C_TAB + HT + 1]
        segT = const_sb[:H, C_SEGT:C_SEGT + HT]

        @block.sync
        def _(sync):
            sync.dma_start(const_sb[:], const_d[:]).then_inc(c_sem, 16)
            for i in range(BPC):
                s = i % 3
                if i >= 3:
                    sync.wait_ge(pe_sem, 5 * (i - 3) + 1)   # A_{i-3} read slot
                sync.dma_start(sts[s][:], stream_d[i]).then_inc(st_sems[s], 16)
            # drain: all output DMAs complete before NEFF end
            for h in range(2):
                for bb in range(2):
                    cnt = len([i for i in range(BPC) if i % 2 == bb])
                    sync.wait_ge(ot_sems[h][bb], 16 * cnt)

        @block.tensor
        def _(tensor):
            tensor.wait_ge(c_sem, 16)
            for i in range(BPC):
                s, pb = i % 3, i % 2
                tensor.wait_ge(st_sems[s], 16 * (i // 3 + 1))
                if i >= 2:
                    tensor.wait_ge(act_sem, 2 * (i - 2) + 1)  # ps_s slot free
                tensor.matmul(ps_ss[pb][:], table, sts[s][:],
                              start=True, stop=True).then_inc(pe_sem, 1)
                tensor.wait_ge(act_sem, 2 * i + 1)            # exp_i done
                tensor.matmul(ps_z[:], seg, ps[pb][:],
                              start=True, stop=True).then_inc(pe_sem, 1)
                tensor.wait_ge(dve_sem, 3 * i + 1)            # recip_i done
                tensor.matmul(ps_r[:], segT, r_sb[:],
                              start=True, stop=True).then_inc(pe_sem, 1)
                tensor.wait_ge(dve_sem, 3 * i + 2)            # mul_i done
                tensor.matmul(ps_os[0][pb][:], vproj[:, 0:128], ps[pb][:],
                              start=True, stop=True).then_inc(pe_sem, 1)
                tensor.matmul(ps_os[1][pb][:], vproj[:, 128:256], ps[pb][:],
                              start=True, stop=True).then_inc(pe_sem, 1)

        @block.scalar
        def _(scalar):
            scalar.wait_ge(c_sem, 16)
            for i in range(BPC):
                pb = ob = i % 2
                scalar.wait_ge(pe_sem, 5 * i + 1)             # A_i done
                scalar.activation(ps[pb][:], ps_ss[pb][:], Exp,
                                  bias=us[:, i:i + 1],
                                  scale=1.0).then_inc(act_sem, 1)
                scalar.wait_ge(pe_sem, 5 * i + 4)             # mm2a_i done
                if i >= 2:
                    scalar.wait_ge(ot_sems[0][ob], 16 * (i // 2))
                scalar.copy(ots[0][ob][:], ps_os[0][pb][:]).then_inc(act_sem, 1)
                scalar.dma_start(out_d[i, 0:128, :],
                                 ots[0][ob][:]).then_inc(ot_sems[0][ob], 16)
                scalar.wait_ge(dve_sem, 3 * i + 3)            # ocopy1_i done
                scalar.dma_start(out_d[i, 128:256, :],
                                 ots[1][ob][:]).then_inc(ot_sems[1][ob], 16)

        @block.vector
        def _(vector):
            for i in range(BPC):
                pb = ob = i % 2
                vector.wait_ge(pe_sem, 5 * i + 2)             # headsum_i done
                vector.reciprocal(r_sb[:], ps_z[:]).then_inc(dve_sem, 1)
                vector.wait_ge(pe_sem, 5 * i + 3)             # bcast_i done
                vector.tensor_mul(ps[pb][:HT, :], ps[pb][:HT, :],
                                  ps_r[:]).then_inc(dve_sem, 1)
                vector.wait_ge(pe_sem, 5 * i + 5)             # mm2b_i done
                if i >= 2:
                    vector.wait_ge(ot_sems[1][ob], 16 * (i // 2))
                vector.tensor_copy(ots[1][ob][:],
                                   ps_os[1][pb][:]).then_inc(dve_sem, 1)
    return nc


def _run(inputs, trace=False):
    from concourse.bass_utils import run_bass_kernel_spmd

    consts, streams = _host_tables(**inputs)
    nc = _build_program()
    in_maps = [
        {"stream": streams[c], "const": consts[c]}
        for c in range(NCORES)
    ]
    res = run_bass_kernel_spmd(nc, in_maps, core_ids=list(range(NCORES)),
                               trace=trace)
    out_full = np.empty((B, S, D), np.float32)
    for c in range(NCORES):
        oc = res.results[c]["out"]  # [BPC, D, S]
        out_full[c * BPC:(c + 1) * BPC] = oc.transpose(0, 2, 1)
    return out_full, res


def kernel(**inputs):
    trace = bool(int(os.environ.get("BASS_KERNEL_TRACE", "0")))
    out, _ = _run(inputs, trace=trace)
    return out


def kernel_profiled(**inputs):
    out, res = _run(inputs, trace=True)
    return out, res


# revision 6
# speedup vs baseline: 2.4105x; 2.4105x over previous
"""Trainium2 Bass kernel for nn_ArrivalTime (sparse attention over 24 timeslots).

Math refactoring (exact, up to fp reassociation):
  query = [user_pref[user], timeslot[hour]] has only 64 distinct user rows and
  24 distinct time rows, so
    scores[n,h,t] = US[b(n), h, t] + TS[hour[n], h, t]
  with tiny host-precomputed tables
    US = (user_q @ k^T) * scale   [64, H*T]   (user_q folds bq)
    TS = (time_q @ k^T) * scale   [24, H*T]
  Masking adds -1e9 where hour_mask==1.  Softmax per head over t (24).
  Output: out[n,:] = attn[n,:] @ vproj + bu, vproj[(h,t),d] = v[h,t,:]@Wu[d,h*HD:]^T.

Device pipeline (per core, transposed layout: tokens on the free dim), one
iteration per batch row (512 tokens):
  PE : ps_s = table^T @ stream      (one-hot hour + mask rows, bf16)
  ACT: p = exp(ps_s + US_b bias)    (row 96 = exp(0) = 1 -> carries bu)
  PE : ps_z = seg^T @ p             (per-head sums)
  ACT: lnz = ln(ps_z); r = exp(-lnz)     (1/Z without the slow DVE reciprocal)
  PE : ps_r = segT^T @ r            (broadcast 1/Z over each head's 24 rows)
  DVE: p[:96] *= ps_r               (normalize)
  PE : ps_o{h} = vproj[:,h]^T @ p   (two 128-col halves; vproj row 96 = bu)
  DVE: copy psum -> sbuf; SYNC triggers the output DMAs.

Sharding: data-parallel over batch, 8 batch rows (= 8 x 512 tokens) per core.
Raw bass (no Tile): this toolchain's walrus allows at most one attached sem
wait per instruction, so all waits are standalone wait_ge ops with manually
counted thresholds.
"""

import os
import numpy as np

B, S, D, H, HD, T = 64, 512, 256, 4, 64, 24
NCORES = 8
BPC = B // NCORES  # batch rows per core
HT = H * T  # 96
K1 = 2 * T  # 48 stream rows: one-hot hour + mask
MASK_NEG = -1.0e9

# packed bf16 constant tensor [HT+1, CWB]: columns
C_VP = 0                    # vproj_ext [97, 256]
C_SEG = C_VP + D            # seg [97, 4]
C_TAB = C_SEG + H           # table (valid rows :K1) [48, 97]
C_SEGT = C_TAB + (HT + 1)   # segT (valid rows :H) [4, 96]
CWB = C_SEGT + HT


def _host_tables(timeslot_embedded, user, hour, hour_mask, user_pref,
                 Wq, bq, Wk, bk, Wv, bv, Wu, bu):
    import ml_dtypes
    f32 = np.float32
    bf16 = ml_dtypes.bfloat16
    ts_e = np.asarray(timeslot_embedded, f32)          # [T, D]
    user = np.asarray(user).astype(np.int64)           # [B]
    hour = np.asarray(hour).astype(np.int64)           # [B, S]
    hour_mask = np.asarray(hour_mask)                  # [B, S, T]
    Wq = np.asarray(Wq, f32); bq = np.asarray(bq, f32)
    Wk = np.asarray(Wk, f32); bk = np.asarray(bk, f32)
    Wv = np.asarray(Wv, f32); bv = np.asarray(bv, f32)
    Wu = np.asarray(Wu, f32); bu = np.asarray(bu, f32)

    Wq_u, Wq_t = Wq[:, :, :D], Wq[:, :, D:]
    k_ = np.einsum('td,hkd->htk', ts_e, Wk) + bk[:, None, :]   # [H,T,HD]
    v_ = np.einsum('td,hkd->htk', ts_e, Wv) + bv[:, None, :]
    time_q = np.einsum('td,hkd->thk', ts_e, Wq_t)              # [T,H,HD]
    upref = np.asarray(user_pref, f32)[user]                   # [B,D]
    user_q = np.einsum('bd,hkd->bhk', upref, Wq_u) + bq[None]  # [B,H,HD]
    scale = f32(1.0 / np.sqrt(HD))
    TS = (np.einsum('thk,hsk->ths', time_q, k_) * scale).reshape(T, HT)
    US = (np.einsum('bhk,hsk->bhs', user_q, k_) * scale).reshape(B, HT)
    vproj = np.einsum('htk,dhk->htd', v_, Wu.reshape(D, H, HD)).reshape(HT, D)

    # table [K1, HT+1]: rows 0..23 TS, rows 24..47 mask additive; col 96 = 0
    table = np.zeros((K1, HT + 1), f32)
    table[:T, :HT] = TS
    table[T:, :HT] = np.tile(np.eye(T, dtype=f32), (1, H)) * f32(MASK_NEG)

    seg = np.zeros((HT + 1, H), f32)
    seg[:HT] = np.repeat(np.eye(H, dtype=f32), T, axis=0)
    segT = np.ascontiguousarray(seg[:HT].T)                    # [H, HT]

    cb = np.zeros((HT + 1, CWB), f32)
    cb[:HT, C_VP:C_VP + D] = vproj
    cb[HT, C_VP:C_VP + D] = bu
    cb[:, C_SEG:C_SEG + H] = seg
    cb[:K1, C_TAB:C_TAB + HT + 1] = table
    cb[:H, C_SEGT:C_SEGT + HT] = segT
    const_bf = cb.astype(bf16)

    us_all = np.zeros((HT + 1, B), f32)
    us_all[:HT, :] = US.T
    us_cols = [np.ascontiguousarray(us_all[:, c * BPC:(c + 1) * BPC])
               for c in range(NCORES)]

    # per-core streams [BPC, K1, S] bf16: one-hot(hour) rows + mask^T rows
    eyeT = np.eye(T, dtype=f32)
    streams = []
    for c in range(NCORES):
        hb = hour[c * BPC:(c + 1) * BPC]                       # [BPC, S]
        mb = hour_mask[c * BPC:(c + 1) * BPC]                  # [BPC, S, T]
        st = np.empty((BPC, K1, S), f32)
        st[:, :T, :] = eyeT[hb].transpose(0, 2, 1)
        st[:, T:, :] = mb.astype(f32).transpose(0, 2, 1)
        streams.append(st.astype(bf16))
    return const_bf, us_cols, streams


def _build_program():
    import concourse.bass as bass
    import concourse.mybir as mybir
    from contextlib import ExitStack

    f32 = mybir.dt.float32
    bf16 = mybir.dt.bfloat16
    nc = bass.Bass("TRN2")
    stream_d = nc.declare_dram_parameter("stream", [BPC, K1, S], bf16,
                                         isOutput=False)
    const_d = nc.declare_dram_parameter("const", [HT + 1, CWB], bf16,
                                        isOutput=False)
    us_d = nc.declare_dram_parameter("usb", [HT + 1, BPC], f32, isOutput=False)
    out_d = nc.declare_dram_parameter("out", [BPC, D, S], f32, isOutput=True)

    Exp = mybir.ActivationFunctionType.Exp
    Ln = mybir.ActivationFunctionType.Ln

    # Engine ticks per iteration i:
    #   PE : A=5i+1  headsum=5i+2  bcast=5i+3  mm2a=5i+4  mm2b=5i+5
    #   ACT: exp1=3i+1  ln=3i+2  expneg=3i+3
    #   DVE: mul=3i+1  oco0=3i+2  oco1=3i+3
    with ExitStack() as ctx:
        ec = ctx.enter_context
        const_sb = ec(nc.sbuf_tensor("const_sb", [HT + 1, CWB], bf16))
        us_sb = ec(nc.sbuf_tensor("us_sb", [HT + 1, BPC], f32))
        sts = [ec(nc.sbuf_tensor(f"st{j}", [K1, S], bf16)) for j in range(3)]
        ps = [ec(nc.sbuf_tensor(f"p{j}", [HT + 1, S], bf16)) for j in range(2)]
        lnz_sb = ec(nc.sbuf_tensor("lnz_sb", [H, S], f32))
        r_sb = ec(nc.sbuf_tensor("r_sb", [H, S], bf16))
        ots = [[ec(nc.sbuf_tensor(f"ot{h}{j}", [128, S], f32))
                for j in range(2)] for h in range(2)]
        ps_ss = [ec(nc.psum_tensor(f"ps_s{j}", [HT + 1, S], f32))
                 for j in range(2)]
        ps_z = ec(nc.psum_tensor("ps_z", [H, S], f32))
        ps_r = ec(nc.psum_tensor("ps_r", [HT, S], f32))
        ps_os = [[ec(nc.psum_tensor(f"ps_o{h}{j}", [128, S], f32))
                  for j in range(2)] for h in range(2)]
        c_sem = ec(nc.semaphore("c_sem"))
        u_sem = ec(nc.semaphore("u_sem"))
        st_sems = [ec(nc.semaphore(f"st_sem{j}")) for j in range(3)]
        pe_sem = ec(nc.semaphore("pe_sem"))
        act_sem = ec(nc.semaphore("act_sem"))
        dve_sem = ec(nc.semaphore("dve_sem"))
        ot_sems = [[ec(nc.semaphore(f"ot_sem{h}{j}")) for j in range(2)]
                   for h in range(2)]
        block = ec(nc.Block())

        vproj = const_sb[:, C_VP:C_VP + D]
        seg = const_sb[:, C_SEG:C_SEG + H]
        table = const_sb[:K1, C_TAB:C_TAB + HT + 1]
        segT = const_sb[:H, C_SEGT:C_SEGT + HT]

        @block.sync
        def _(sync):
            sync.dma_start(const_sb[:], const_d[:]).then_inc(c_sem, 16)
            sync.dma_start(us_sb[:], us_d[:]).then_inc(u_sem, 16)
            for i in range(3):
                sync.dma_start(sts[i][:], stream_d[i]).then_inc(st_sems[i], 16)
            for i in range(BPC):
                if i + 3 < BPC:
                    s = (i + 3) % 3
                    sync.wait_ge(pe_sem, 5 * i + 1)   # A_i freed slot i%3
                    sync.dma_start(sts[s][:],
                                   stream_d[i + 3]).then_inc(st_sems[s], 16)
                ob = i % 2
                sync.wait_ge(dve_sem, 3 * i + 2)      # oco0_i done
                sync.dma_start(out_d[i, 0:128, :],
                               ots[0][ob][:]).then_inc(ot_sems[0][ob], 16)
                sync.wait_ge(dve_sem, 3 * i + 3)      # oco1_i done
                sync.dma_start(out_d[i, 128:256, :],
                               ots[1][ob][:]).then_inc(ot_sems[1][ob], 16)
            for h in range(2):
                for bb in range(2):
                    cnt = len([i for i in range(BPC) if i % 2 == bb])
                    sync.wait_ge(ot_sems[h][bb], 16 * cnt)

        @block.tensor
        def _(tensor):
            tensor.wait_ge(c_sem, 16)
            for i in range(BPC):
                s, pb = i % 3, i % 2
                tensor.wait_ge(st_sems[s], 16 * (i // 3 + 1))
                if i >= 2:
                    tensor.wait_ge(act_sem, 3 * (i - 2) + 1)  # ps_s slot free
                tensor.matmul(ps_ss[pb][:], table, sts[s][:],
                              start=True, stop=True).then_inc(pe_sem, 1)
                tensor.wait_ge(act_sem, 3 * i + 1)            # exp1_i done
                tensor.matmul(ps_z[:], seg, ps[pb][:],
                              start=True, stop=True).then_inc(pe_sem, 1)
                tensor.wait_ge(act_sem, 3 * i + 3)            # expneg_i done
                tensor.matmul(ps_r[:], segT, r_sb[:],
                              start=True, stop=True).then_inc(pe_sem, 1)
                tensor.wait_ge(dve_sem, 3 * i + 1)            # mul_i done
                tensor.matmul(ps_os[0][pb][:], vproj[:, 0:128], ps[pb][:],
                              start=True, stop=True).then_inc(pe_sem, 1)
                tensor.matmul(ps_os[1][pb][:], vproj[:, 128:256], ps[pb][:],
                              start=True, stop=True).then_inc(pe_sem, 1)

        @block.scalar
        def _(scalar):
            scalar.wait_ge(u_sem, 16)
            for i in range(BPC):
                pb = i % 2
                scalar.wait_ge(pe_sem, 5 * i + 1)             # A_i done
                scalar.activation(ps[pb][:], ps_ss[pb][:], Exp,
                                  bias=us_sb[:, i:i + 1],
                                  scale=1.0).then_inc(act_sem, 1)
                scalar.wait_ge(pe_sem, 5 * i + 2)             # headsum_i done
                scalar.activation(lnz_sb[:], ps_z[:], Ln).then_inc(act_sem, 1)
                scalar.activation(r_sb[:], lnz_sb[:], Exp,
                                  scale=-1.0).then_inc(act_sem, 1)

        @block.vector
        def _(vector):
            for i in range(BPC):
                pb = ob = i % 2
                vector.wait_ge(pe_sem, 5 * i + 3)             # bcast_i done
                vector.tensor_mul(ps[pb][:HT, :], ps[pb][:HT, :],
                                  ps_r[:]).then_inc(dve_sem, 1)
                vector.wait_ge(pe_sem, 5 * i + 4)             # mm2a_i done
                if i >= 2:
                    vector.wait_ge(ot_sems[0][ob], 16 * (i // 2))
                vector.tensor_copy(ots[0][ob][:],
                                   ps_os[0][pb][:]).then_inc(dve_sem, 1)
                vector.wait_ge(pe_sem, 5 * i + 5)             # mm2b_i done
                if i >= 2:
                    vector.wait_ge(ot_sems[1][ob], 16 * (i // 2))
                vector.tensor_copy(ots[1][ob][:],
                                   ps_os[1][pb][:]).then_inc(dve_sem, 1)
    return nc


def _run(inputs, trace=False):
    import sys
    if "/opt/trn_rl_repo" not in sys.path:
        sys.path.insert(0, "/opt/trn_rl_repo")
    from concourse.bass_utils import run_bass_kernel_spmd

    const_bf, us_cols, streams = _host_tables(**inputs)
    nc = _build_program()
    in_maps = [
        {"stream": streams[c], "const": const_bf, "usb": us_cols[c]}
        for c in range(NCORES)
    ]
    res = run_bass_kernel_spmd(nc, in_maps, core_ids=list(range(NCORES)),
                               trace=trace)
    out_full = np.empty((B, S, D), np.float32)
    for c in range(NCORES):
        oc = res.results[c]["out"]  # [BPC, D, S]
        out_full[c * BPC:(c + 1) * BPC] = oc.transpose(0, 2, 1)
    return out_full, res


def kernel(**inputs):
    trace = bool(int(os.environ.get("BASS_KERNEL_TRACE", "0")))
    out, _ = _run(inputs, trace=trace)
    return out


def kernel_profiled(**inputs):
    out, res = _run(inputs, trace=True)
    return out, res


# revision 7
# speedup vs baseline: 2.8444x; 1.1800x over previous
"""Trainium2 Bass kernel for nn_ArrivalTime (sparse attention over 24 timeslots).

Math refactoring (exact, up to fp reassociation):
  query = [user_pref[user], timeslot[hour]] has only 64 distinct user rows and
  24 distinct time rows, so
    scores[n,h,t] = US[b(n), h, t] + TS[hour[n], h, t]
  with tiny host-precomputed tables
    US = (user_q @ k^T) * scale   [64, H*T]   (user_q folds bq)
    TS = (time_q @ k^T) * scale   [24, H*T]
  Masking adds -1e9 where hour_mask==1.  Softmax per head over t (24).
  Output: out[n,:] = attn[n,:] @ vproj + bu, vproj[(h,t),d] = v[h,t,:]@Wu[d,h*HD:]^T.

Device pipeline (per core, transposed layout: tokens on the free dim), one
iteration per batch row (512 tokens):
  PE : ps_s = table^T @ stream      (one-hot hour + mask rows, bf16)
  ACT: p = exp(ps_s + US_b bias)    (row 96 = exp(0) = 1 -> carries bu)
  PE : ps_z = seg^T @ p             (per-head sums)
  ACT: lnz = ln(ps_z); r = exp(-lnz)     (1/Z without the slow DVE reciprocal)
  PE : ps_r = segT^T @ r            (broadcast 1/Z over each head's 24 rows)
  DVE: p[:96] *= ps_r               (normalize)
  PE : ps_o{h} = vproj[:,h]^T @ p   (two 128-col halves; vproj row 96 = bu)
  DVE: copy psum -> sbuf; SYNC triggers the output DMAs.

Sharding: data-parallel over batch, 8 batch rows (= 8 x 512 tokens) per core.
Raw bass (no Tile): this toolchain's walrus allows at most one attached sem
wait per instruction, so all waits are standalone wait_ge ops with manually
counted thresholds.
"""

import os
import numpy as np

B, S, D, H, HD, T = 64, 512, 256, 4, 64, 24
NCORES = 8
BPC = B // NCORES  # batch rows per core
HT = H * T  # 96
K1 = 2 * T  # 48 stream rows: one-hot hour + mask
MASK_NEG = -1.0e9

# packed bf16 constant tensor [HT+1, CWB]: columns
C_VP = 0                    # vproj_ext [97, 256]
C_SEG = C_VP + D            # seg [97, 4]
C_TAB = C_SEG + H           # table (valid rows :K1) [48, 97]
C_SEGT = C_TAB + (HT + 1)   # segT (valid rows :H) [4, 96]
CWB = C_SEGT + HT


def _host_tables(timeslot_embedded, user, hour, hour_mask, user_pref,
                 Wq, bq, Wk, bk, Wv, bv, Wu, bu):
    import ml_dtypes
    f32 = np.float32
    bf16 = ml_dtypes.bfloat16
    ts_e = np.asarray(timeslot_embedded, f32)          # [T, D]
    user = np.asarray(user).astype(np.int64)           # [B]
    hour = np.asarray(hour).astype(np.int64)           # [B, S]
    hour_mask = np.asarray(hour_mask)                  # [B, S, T]
    Wq = np.asarray(Wq, f32); bq = np.asarray(bq, f32)
    Wk = np.asarray(Wk, f32); bk = np.asarray(bk, f32)
    Wv = np.asarray(Wv, f32); bv = np.asarray(bv, f32)
    Wu = np.asarray(Wu, f32); bu = np.asarray(bu, f32)

    Wq_u, Wq_t = Wq[:, :, :D], Wq[:, :, D:]
    k_ = np.einsum('td,hkd->htk', ts_e, Wk) + bk[:, None, :]   # [H,T,HD]
    v_ = np.einsum('td,hkd->htk', ts_e, Wv) + bv[:, None, :]
    time_q = np.einsum('td,hkd->thk', ts_e, Wq_t)              # [T,H,HD]
    upref = np.asarray(user_pref, f32)[user]                   # [B,D]
    user_q = np.einsum('bd,hkd->bhk', upref, Wq_u) + bq[None]  # [B,H,HD]
    scale = f32(1.0 / np.sqrt(HD))
    TS = (np.einsum('thk,hsk->ths', time_q, k_) * scale).reshape(T, HT)
    US = (np.einsum('bhk,hsk->bhs', user_q, k_) * scale).reshape(B, HT)
    vproj = np.einsum('htk,dhk->htd', v_, Wu.reshape(D, H, HD)).reshape(HT, D)

    # table [K1, HT+1]: rows 0..23 TS, rows 24..47 mask additive; col 96 = 0
    table = np.zeros((K1, HT + 1), f32)
    table[:T, :HT] = TS
    table[T:, :HT] = np.tile(np.eye(T, dtype=f32), (1, H)) * f32(MASK_NEG)

    seg = np.zeros((HT + 1, H), f32)
    seg[:HT] = np.repeat(np.eye(H, dtype=f32), T, axis=0)
    segT = np.ascontiguousarray(seg[:HT].T)                    # [H, HT]

    cb = np.zeros((HT + 1, CWB), f32)
    cb[:HT, C_VP:C_VP + D] = vproj
    cb[HT, C_VP:C_VP + D] = bu
    cb[:, C_SEG:C_SEG + H] = seg
    cb[:K1, C_TAB:C_TAB + HT + 1] = table
    cb[:H, C_SEGT:C_SEGT + HT] = segT
    const_bf = cb.astype(bf16)

    us_all = np.zeros((HT + 1, B), f32)
    us_all[:HT, :] = US.T
    us_cols = [np.ascontiguousarray(us_all[:, c * BPC:(c + 1) * BPC])
               for c in range(NCORES)]

    # per-core streams [BPC, K1, S] bf16: one-hot(hour) rows + mask^T rows
    eyeT = np.eye(T, dtype=f32)
    streams = []
    for c in range(NCORES):
        hb = hour[c * BPC:(c + 1) * BPC]                       # [BPC, S]
        mb = hour_mask[c * BPC:(c + 1) * BPC]                  # [BPC, S, T]
        st = np.empty((BPC, K1, S), f32)
        st[:, :T, :] = eyeT[hb].transpose(0, 2, 1)
        st[:, T:, :] = mb.astype(f32).transpose(0, 2, 1)
        streams.append(st.astype(bf16))
    return const_bf, us_cols, streams


def _build_program():
    import concourse.bass as bass
    import concourse.mybir as mybir
    from contextlib import ExitStack

    f32 = mybir.dt.float32
    bf16 = mybir.dt.bfloat16
    nc = bass.Bass("TRN2")
    stream_d = nc.declare_dram_parameter("stream", [BPC, K1, S], bf16,
                                         isOutput=False)
    const_d = nc.declare_dram_parameter("const", [HT + 1, CWB], bf16,
                                        isOutput=False)
    us_d = nc.declare_dram_parameter("usb", [HT + 1, BPC], f32, isOutput=False)
    out_d = nc.declare_dram_parameter("out", [BPC, D, S], f32, isOutput=True)

    Exp = mybir.ActivationFunctionType.Exp
    Ln = mybir.ActivationFunctionType.Ln

    # Software-pipelined engine programs.  PE emission order per block j:
    #   bc_{j-1} | A_{j+1} | hs_j | mm2a_{j-1} mm2b_{j-1}
    # so iteration j's ACT chain (exp1/ln/expneg) overlaps iteration j-1's
    # matmul tail.  ACT ticks: exp1_i=3i+1 ln=3i+2 expneg=3i+3.
    # DVE ticks: mul_i=2i+1 ocopy_i=2i+2.  PE ticks recorded at emission.
    with ExitStack() as ctx:
        ec = ctx.enter_context
        const_sb = ec(nc.sbuf_tensor("const_sb", [HT + 1, CWB], bf16))
        us_sb = ec(nc.sbuf_tensor("us_sb", [HT + 1, BPC], f32))
        sts = [ec(nc.sbuf_tensor(f"st{j}", [K1, S], bf16)) for j in range(3)]
        ps = [ec(nc.sbuf_tensor(f"p{j}", [HT + 1, S], bf16)) for j in range(3)]
        lnz_sb = ec(nc.sbuf_tensor("lnz_sb", [H, S], f32))
        r_sb = ec(nc.sbuf_tensor("r_sb", [H, S], bf16))
        ots = [ec(nc.sbuf_tensor(f"ot{j}", [128, 2 * S], f32))
               for j in range(2)]
        ps_ss = [ec(nc.psum_tensor(f"ps_s{j}", [HT + 1, S], f32))
                 for j in range(2)]
        ps_z = ec(nc.psum_tensor("ps_z", [H, S], f32))
        ps_r = ec(nc.psum_tensor("ps_r", [HT, S], f32))
        ps_os = [ec(nc.psum_tensor(f"ps_o{j}", [128, 2 * S], f32))
                 for j in range(2)]
        c_sem = ec(nc.semaphore("c_sem"))
        u_sem = ec(nc.semaphore("u_sem"))
        st_sems = [ec(nc.semaphore(f"st_sem{j}")) for j in range(3)]
        pe_sem = ec(nc.semaphore("pe_sem"))
        act_sem = ec(nc.semaphore("act_sem"))
        dve_sem = ec(nc.semaphore("dve_sem"))
        ot_sems = [ec(nc.semaphore(f"ot_sem{j}")) for j in range(2)]
        block = ec(nc.Block())

        vproj = const_sb[:, C_VP:C_VP + D]
        seg = const_sb[:, C_SEG:C_SEG + H]
        table = const_sb[:K1, C_TAB:C_TAB + HT + 1]
        segT = const_sb[:H, C_SEGT:C_SEGT + HT]

        pe_tick = {}
        pe_cnt = [0]

        @block.tensor
        def _(tensor):
            def mm(key, out, lhsT, rhs):
                tensor.matmul(out, lhsT, rhs,
                              start=True, stop=True).then_inc(pe_sem, 1)
                pe_cnt[0] += 1
                pe_tick[key] = pe_cnt[0]

            tensor.wait_ge(c_sem, 16)
            tensor.wait_ge(st_sems[0], 16)
            mm(('A', 0), ps_ss[0][:], table, sts[0][:])
            for j in range(BPC + 1):
                if 1 <= j:                      # bc_{j-1}
                    i = j - 1
                    tensor.wait_ge(act_sem, 3 * i + 3)   # expneg_i done
                    mm(('bc', i), ps_r[:], segT, r_sb[:])
                if j + 1 < BPC:                 # A_{j+1}
                    i = j + 1
                    tensor.wait_ge(st_sems[i % 3], 16 * (i // 3 + 1))
                    mm(('A', i), ps_ss[i % 2][:], table, sts[i % 3][:])
                if j < BPC:                     # hs_j
                    tensor.wait_ge(act_sem, 3 * j + 1)   # exp1_j done
                    mm(('hs', j), ps_z[:], seg, ps[j % 3][:])
                if 1 <= j:                      # mm2_{j-1}
                    i = j - 1
                    tensor.wait_ge(dve_sem, 2 * i + 1)   # mul_i done
                    mm(('m2a', i), ps_os[i % 2][:, 0:S],
                       vproj[:, 0:128], ps[i % 3][:])
                    mm(('m2b', i), ps_os[i % 2][:, S:2 * S],
                       vproj[:, 128:256], ps[i % 3][:])

        @block.scalar
        def _(scalar):
            scalar.wait_ge(u_sem, 16)
            for i in range(BPC):
                scalar.wait_ge(pe_sem, pe_tick[('A', i)])
                scalar.activation(ps[i % 3][:], ps_ss[i % 2][:], Exp,
                                  bias=us_sb[:, i:i + 1],
                                  scale=1.0).then_inc(act_sem, 1)
                scalar.wait_ge(pe_sem, pe_tick[('hs', i)])
                scalar.activation(lnz_sb[:], ps_z[:], Ln).then_inc(act_sem, 1)
                scalar.activation(r_sb[:], lnz_sb[:], Exp,
                                  scale=-1.0).then_inc(act_sem, 1)

        @block.vector
        def _(vector):
            for i in range(BPC):
                vector.wait_ge(pe_sem, pe_tick[('bc', i)])
                vector.tensor_mul(ps[i % 3][:HT, :], ps[i % 3][:HT, :],
                                  ps_r[:]).then_inc(dve_sem, 1)
                vector.wait_ge(pe_sem, pe_tick[('m2b', i)])
                if i >= 2:
                    vector.wait_ge(ot_sems[i % 2], 16 * (i // 2))
                vector.tensor_copy(ots[i % 2][:],
                                   ps_os[i % 2][:]).then_inc(dve_sem, 1)

        @block.sync
        def _(sync):
            sync.dma_start(const_sb[:], const_d[:]).then_inc(c_sem, 16)
            sync.dma_start(us_sb[:], us_d[:]).then_inc(u_sem, 16)
            for i in range(min(3, BPC)):
                sync.dma_start(sts[i][:], stream_d[i]).then_inc(st_sems[i], 16)
            for i in range(BPC):
                if i + 3 < BPC:
                    s = (i + 3) % 3
                    sync.wait_ge(pe_sem, pe_tick[('A', i)])
                    sync.dma_start(sts[s][:],
                                   stream_d[i + 3]).then_inc(st_sems[s], 16)
                sync.wait_ge(dve_sem, 2 * i + 2)      # ocopy_i done
                dest = out_d[i, :, :].rearrange("(h p) s -> p h s", h=2)
                src = ots[i % 2][:, :].rearrange("p (h s) -> p h s", h=2)
                sync.dma_start(dest, src).then_inc(ot_sems[i % 2], 16)
            for bb in range(2):
                cnt = len([i for i in range(BPC) if i % 2 == bb])
                sync.wait_ge(ot_sems[bb], 16 * cnt)
    return nc


def _run(inputs, trace=False):
    import sys
    if "/opt/trn_rl_repo" not in sys.path:
        sys.path.insert(0, "/opt/trn_rl_repo")
    from concourse.bass_utils import run_bass_kernel_spmd

    const_bf, us_cols, streams = _host_tables(**inputs)
    nc = _build_program()
    in_maps = [
        {"stream": streams[c], "const": const_bf, "usb": us_cols[c]}
        for c in range(NCORES)
    ]
    res = run_bass_kernel_spmd(nc, in_maps, core_ids=list(range(NCORES)),
                               trace=trace)
    out_full = np.empty((B, S, D), np.float32)
    for c in range(NCORES):
        oc = res.results[c]["out"]  # [BPC, D, S]
        out_full[c * BPC:(c + 1) * BPC] = oc.transpose(0, 2, 1)
    return out_full, res


def kernel(**inputs):
    trace = bool(int(os.environ.get("BASS_KERNEL_TRACE", "0")))
    out, _ = _run(inputs, trace=trace)
    return out


def kernel_profiled(**inputs):
    out, res = _run(inputs, trace=True)
    return out, res


# revision 8
# speedup vs baseline: 2.8676x; 1.0082x over previous
"""Trainium2 Bass kernel for nn_ArrivalTime (sparse attention over 24 timeslots).

Math refactoring (exact, up to fp reassociation):
  query = [user_pref[user], timeslot[hour]] has only 64 distinct user rows and
  24 distinct time rows, so
    scores[n,h,t] = US[b(n), h, t] + TS[hour[n], h, t]
  with tiny host-precomputed tables
    US = (user_q @ k^T) * scale   [64, H*T]   (user_q folds bq)
    TS = (time_q @ k^T) * scale   [24, H*T]
  Masking adds -1e9 where hour_mask==1.  Softmax per head over t (24).
  Output: out[n,:] = attn[n,:] @ vproj + bu, vproj[(h,t),d] = v[h,t,:]@Wu[d,h*HD:]^T.

Device pipeline (per core, transposed layout: tokens on the free dim), one
iteration per batch row (512 tokens):
  PE : ps_s = table^T @ stream      (one-hot hour + mask rows, bf16)
  ACT: p = exp(ps_s + US_b bias)    (row 96 = exp(0) = 1 -> carries bu)
  PE : ps_z = seg^T @ p             (per-head sums)
  ACT: lnz = ln(ps_z); r = exp(-lnz)     (1/Z without the slow DVE reciprocal)
  PE : ps_r = segT^T @ r            (broadcast 1/Z over each head's 24 rows)
  DVE: p[:96] *= ps_r               (normalize)
  PE : ps_o{h} = vproj[:,h]^T @ p   (two 128-col halves; vproj row 96 = bu)
  DVE: copy psum -> sbuf; SYNC triggers the output DMAs.

Sharding: data-parallel over batch, 8 batch rows (= 8 x 512 tokens) per core.
Raw bass (no Tile): this toolchain's walrus allows at most one attached sem
wait per instruction, so all waits are standalone wait_ge ops with manually
counted thresholds.
"""

import os
import numpy as np

B, S, D, H, HD, T = 64, 512, 256, 4, 64, 24
NCORES = 8
BPC = B // NCORES  # batch rows per core
HT = H * T  # 96
K1 = 2 * T  # 48 stream rows: one-hot hour + mask
MASK_NEG = -1.0e9

# packed bf16 constant tensor [HT+1, CWB]: columns
C_VP = 0                    # vproj_ext [97, 256]
C_SEG = C_VP + D            # seg [97, 4]
C_TAB = C_SEG + H           # table (valid rows :K1) [48, 97]
C_SEGT = C_TAB + (HT + 1)   # segT (valid rows :H) [4, 96]
CWB = C_SEGT + HT


def _host_tables(timeslot_embedded, user, hour, hour_mask, user_pref,
                 Wq, bq, Wk, bk, Wv, bv, Wu, bu):
    import ml_dtypes
    f32 = np.float32
    bf16 = ml_dtypes.bfloat16
    ts_e = np.asarray(timeslot_embedded, f32)          # [T, D]
    user = np.asarray(user).astype(np.int64)           # [B]
    hour = np.asarray(hour).astype(np.int64)           # [B, S]
    hour_mask = np.asarray(hour_mask)                  # [B, S, T]
    Wq = np.asarray(Wq, f32); bq = np.asarray(bq, f32)
    Wk = np.asarray(Wk, f32); bk = np.asarray(bk, f32)
    Wv = np.asarray(Wv, f32); bv = np.asarray(bv, f32)
    Wu = np.asarray(Wu, f32); bu = np.asarray(bu, f32)

    Wq_u, Wq_t = Wq[:, :, :D], Wq[:, :, D:]
    k_ = np.einsum('td,hkd->htk', ts_e, Wk) + bk[:, None, :]   # [H,T,HD]
    v_ = np.einsum('td,hkd->htk', ts_e, Wv) + bv[:, None, :]
    time_q = np.einsum('td,hkd->thk', ts_e, Wq_t)              # [T,H,HD]
    upref = np.asarray(user_pref, f32)[user]                   # [B,D]
    user_q = np.einsum('bd,hkd->bhk', upref, Wq_u) + bq[None]  # [B,H,HD]
    scale = f32(1.0 / np.sqrt(HD))
    TS = (np.einsum('thk,hsk->ths', time_q, k_) * scale).reshape(T, HT)
    US = (np.einsum('bhk,hsk->bhs', user_q, k_) * scale).reshape(B, HT)
    vproj = np.einsum('htk,dhk->htd', v_, Wu.reshape(D, H, HD)).reshape(HT, D)

    # table [K1, HT+1]: rows 0..23 TS, rows 24..47 mask additive; col 96 = 0
    table = np.zeros((K1, HT + 1), f32)
    table[:T, :HT] = TS
    table[T:, :HT] = np.tile(np.eye(T, dtype=f32), (1, H)) * f32(MASK_NEG)

    seg = np.zeros((HT + 1, H), f32)
    seg[:HT] = np.repeat(np.eye(H, dtype=f32), T, axis=0)
    segT = np.ascontiguousarray(seg[:HT].T)                    # [H, HT]

    cb = np.zeros((HT + 1, CWB), f32)
    cb[:HT, C_VP:C_VP + D] = vproj
    cb[HT, C_VP:C_VP + D] = bu
    cb[:, C_SEG:C_SEG + H] = seg
    cb[:K1, C_TAB:C_TAB + HT + 1] = table
    cb[:H, C_SEGT:C_SEGT + HT] = segT
    const_bf = cb.astype(bf16)

    us_all = np.zeros((HT + 1, B), f32)
    us_all[:HT, :] = US.T
    us_cols = [np.ascontiguousarray(us_all[:, c * BPC:(c + 1) * BPC])
               for c in range(NCORES)]

    # per-core streams [BPC, K1, S] bf16: one-hot(hour) rows + mask^T rows
    eyeT = np.eye(T, dtype=f32)
    streams = []
    for c in range(NCORES):
        hb = hour[c * BPC:(c + 1) * BPC]                       # [BPC, S]
        mb = hour_mask[c * BPC:(c + 1) * BPC]                  # [BPC, S, T]
        st = np.empty((BPC, K1, S), f32)
        st[:, :T, :] = eyeT[hb].transpose(0, 2, 1)
        st[:, T:, :] = mb.astype(f32).transpose(0, 2, 1)
        streams.append(st.astype(bf16))
    return const_bf, us_cols, streams


def _build_program():
    import concourse.bass as bass
    import concourse.mybir as mybir
    from contextlib import ExitStack

    f32 = mybir.dt.float32
    bf16 = mybir.dt.bfloat16
    nc = bass.Bass("TRN2")
    stream_d = nc.declare_dram_parameter("stream", [BPC, K1, S], bf16,
                                         isOutput=False)
    const_d = nc.declare_dram_parameter("const", [HT + 1, CWB], bf16,
                                        isOutput=False)
    us_d = nc.declare_dram_parameter("usb", [HT + 1, BPC], f32, isOutput=False)
    out_d = nc.declare_dram_parameter("out", [BPC, D, S], f32, isOutput=True)

    Exp = mybir.ActivationFunctionType.Exp
    Ln = mybir.ActivationFunctionType.Ln

    # Deep software pipeline: stage lags keep the PE matmul stream free of
    # same-iteration ACT/DVE dependencies (stalls reset the PE clock ramp).
    # PE block j emits: bc_{j-2} | mm2ab_{j-3} | hs_j | A_{j+1}.
    # ACT ticks: exp1_i=3i+1 ln=3i+2 expneg=3i+3.
    # DVE ticks: mul_i=2i+1 ocopy_i=2i+2.  PE ticks recorded at emission.
    with ExitStack() as ctx:
        ec = ctx.enter_context
        const_sb = ec(nc.sbuf_tensor("const_sb", [HT + 1, CWB], bf16))
        us_sb = ec(nc.sbuf_tensor("us_sb", [HT + 1, BPC], f32))
        sts = [ec(nc.sbuf_tensor(f"st{j}", [K1, S], bf16)) for j in range(4)]
        ps = [ec(nc.sbuf_tensor(f"p{j}", [HT + 1, S], bf16)) for j in range(5)]
        lnz_sb = ec(nc.sbuf_tensor("lnz_sb", [H, S], f32))
        r_sbs = [ec(nc.sbuf_tensor(f"r_sb{j}", [H, S], bf16)) for j in range(3)]
        ots = [ec(nc.sbuf_tensor(f"ot{j}", [128, 2 * S], f32))
               for j in range(2)]
        ps_s = ec(nc.psum_tensor("ps_s", [HT + 1, S], f32))
        ps_zs = [ec(nc.psum_tensor(f"ps_z{j}", [H, S], f32)) for j in range(2)]
        ps_r = ec(nc.psum_tensor("ps_r", [HT, S], f32))
        ps_os = [ec(nc.psum_tensor(f"ps_o{j}", [128, 2 * S], f32))
                 for j in range(2)]
        c_sem = ec(nc.semaphore("c_sem"))
        u_sem = ec(nc.semaphore("u_sem"))
        st_sems = [ec(nc.semaphore(f"st_sem{j}")) for j in range(4)]
        pe_sem = ec(nc.semaphore("pe_sem"))
        act_sem = ec(nc.semaphore("act_sem"))
        dve_sem = ec(nc.semaphore("dve_sem"))
        ot_sems = [ec(nc.semaphore(f"ot_sem{j}")) for j in range(2)]
        block = ec(nc.Block())

        vproj = const_sb[:, C_VP:C_VP + D]
        seg = const_sb[:, C_SEG:C_SEG + H]
        table = const_sb[:K1, C_TAB:C_TAB + HT + 1]
        segT = const_sb[:H, C_SEGT:C_SEGT + HT]

        NST = 4   # stream buffers
        pe_tick = {}
        pe_cnt = [0]

        @block.tensor
        def _(tensor):
            def mm(key, out, lhsT, rhs):
                tensor.matmul(out, lhsT, rhs,
                              start=True, stop=True).then_inc(pe_sem, 1)
                pe_cnt[0] += 1
                pe_tick[key] = pe_cnt[0]

            tensor.wait_ge(c_sem, 16)
            tensor.wait_ge(st_sems[0], 16)
            mm(('A', 0), ps_s[:], table, sts[0][:])
            for j in range(BPC + 3):
                if 0 <= j - 2 < BPC:            # bc_{j-2}
                    i = j - 2
                    tensor.wait_ge(act_sem, 3 * i + 3)   # expneg_i done
                    if i >= 1:
                        tensor.wait_ge(dve_sem, 2 * (i - 1) + 1)  # mul_{i-1}
                    mm(('bc', i), ps_r[:], segT, r_sbs[i % 3][:])
                if 0 <= j - 3 < BPC:            # mm2_{j-3}
                    i = j - 3
                    tensor.wait_ge(dve_sem, 2 * i + 1)   # mul_i done
                    mm(('m2a', i), ps_os[i % 2][:, 0:S],
                       vproj[:, 0:128], ps[i % 5][:])
                    mm(('m2b', i), ps_os[i % 2][:, S:2 * S],
                       vproj[:, 128:256], ps[i % 5][:])
                if j < BPC:                     # hs_j
                    tensor.wait_ge(act_sem, 3 * j + 1)   # exp1_j done
                    mm(('hs', j), ps_zs[j % 2][:], seg, ps[j % 5][:])
                if j + 1 < BPC:                 # A_{j+1}
                    i = j + 1
                    tensor.wait_ge(st_sems[i % NST], 16 * (i // NST + 1))
                    if i >= 1:
                        tensor.wait_ge(act_sem, 3 * (i - 1) + 1)  # exp1_{i-1}
                    mm(('A', i), ps_s[:], table, sts[i % NST][:])

        @block.scalar
        def _(scalar):
            scalar.wait_ge(u_sem, 16)
            for i in range(BPC):
                scalar.wait_ge(pe_sem, pe_tick[('A', i)])
                scalar.activation(ps[i % 5][:], ps_s[:], Exp,
                                  bias=us_sb[:, i:i + 1],
                                  scale=1.0).then_inc(act_sem, 1)
                scalar.wait_ge(pe_sem, pe_tick[('hs', i)])
                scalar.activation(lnz_sb[:],
                                  ps_zs[i % 2][:], Ln).then_inc(act_sem, 1)
                scalar.activation(r_sbs[i % 3][:], lnz_sb[:], Exp,
                                  scale=-1.0).then_inc(act_sem, 1)

        @block.vector
        def _(vector):
            for i in range(BPC):
                vector.wait_ge(pe_sem, pe_tick[('bc', i)])
                vector.tensor_mul(ps[i % 5][:HT, :], ps[i % 5][:HT, :],
                                  ps_r[:]).then_inc(dve_sem, 1)
                vector.wait_ge(pe_sem, pe_tick[('m2b', i)])
                if i >= 2:
                    vector.wait_ge(ot_sems[i % 2], 16 * (i // 2))
                vector.tensor_copy(ots[i % 2][:],
                                   ps_os[i % 2][:]).then_inc(dve_sem, 1)

        @block.sync
        def _(sync):
            sync.dma_start(const_sb[:], const_d[:]).then_inc(c_sem, 16)
            for i in range(min(NST, BPC)):
                sync.dma_start(sts[i][:], stream_d[i]).then_inc(st_sems[i], 16)
            for i in range(BPC):
                if i + NST < BPC:
                    s = (i + NST) % NST
                    sync.wait_ge(pe_sem, pe_tick[('A', i)])
                    sync.dma_start(sts[s][:],
                                   stream_d[i + NST]).then_inc(st_sems[s], 16)
                sync.wait_ge(dve_sem, 2 * i + 2)      # ocopy_i done
                dest = out_d[i, :, :].rearrange("(h p) s -> p h s", h=2)
                src = ots[i % 2][:, :].rearrange("p (h s) -> p h s", h=2)
                sync.dma_start(dest, src).then_inc(ot_sems[i % 2], 16)
            for bb in range(2):
                cnt = len([i for i in range(BPC) if i % 2 == bb])
                sync.wait_ge(ot_sems[bb], 16 * cnt)

        @block.gpsimd
        def _(gpsimd):
            gpsimd.dma_start(us_sb[:], us_d[:]).then_inc(u_sem, 16)
    return nc


def _run(inputs, trace=False):
    import sys
    if "/opt/trn_rl_repo" not in sys.path:
        sys.path.insert(0, "/opt/trn_rl_repo")
    from concourse.bass_utils import run_bass_kernel_spmd

    const_bf, us_cols, streams = _host_tables(**inputs)
    nc = _build_program()
    in_maps = [
        {"stream": streams[c], "const": const_bf, "usb": us_cols[c]}
        for c in range(NCORES)
    ]
    res = run_bass_kernel_spmd(nc, in_maps, core_ids=list(range(NCORES)),
                               trace=trace)
    out_full = np.empty((B, S, D), np.float32)
    for c in range(NCORES):
        oc = res.results[c]["out"]  # [BPC, D, S]
        out_full[c * BPC:(c + 1) * BPC] = oc.transpose(0, 2, 1)
    return out_full, res


def kernel(**inputs):
    trace = bool(int(os.environ.get("BASS_KERNEL_TRACE", "0")))
    out, _ = _run(inputs, trace=trace)
    return out


def kernel_profiled(**inputs):
    out, res = _run(inputs, trace=True)
    return out, res


# revision 10
# speedup vs baseline: 2.9515x; 1.0292x over previous
"""Trainium2 Bass kernel for nn_ArrivalTime (sparse attention over 24 timeslots).

Math refactoring (exact, up to fp reassociation):
  query = [user_pref[user], timeslot[hour]] has only 64 distinct user rows and
  24 distinct time rows, so
    scores[n,h,t] = US[b(n), h, t] + TS[hour[n], h, t]
  with tiny host-precomputed tables
    US = (user_q @ k^T) * scale   [64, H*T]   (user_q folds bq)
    TS = (time_q @ k^T) * scale   [24, H*T]
  Masking adds -1e9 where hour_mask==1.  Softmax per head over t (24).
  Output: out[n,:] = attn[n,:] @ vproj + bu, vproj[(h,t),d] = v[h,t,:]@Wu[d,h*HD:]^T.

Device pipeline (per core, transposed layout: tokens on the free dim), one
iteration per batch row (512 tokens):
  PE : ps_s = table^T @ stream      (one-hot hour + mask rows, bf16)
  ACT: p = exp(ps_s + US_b bias)    (row 96 = exp(0) = 1 -> carries bu)
  PE : ps_z = seg^T @ p             (per-head sums)
  ACT: lnz = ln(ps_z); r = exp(-lnz)     (1/Z without the slow DVE reciprocal)
  PE : ps_r = segT^T @ r            (broadcast 1/Z over each head's 24 rows)
  DVE: p[:96] *= ps_r               (normalize)
  PE : ps_o{h} = vproj[:,h]^T @ p   (two 128-col halves; vproj row 96 = bu)
  DVE: copy psum -> sbuf; SYNC triggers the output DMAs.

Sharding: data-parallel over batch, 8 batch rows (= 8 x 512 tokens) per core.
Raw bass (no Tile): this toolchain's walrus allows at most one attached sem
wait per instruction, so all waits are standalone wait_ge ops with manually
counted thresholds.
"""

import os
import numpy as np

B, S, D, H, HD, T = 64, 512, 256, 4, 64, 24
NCORES = 8
BPC = B // NCORES  # batch rows per core
HT = H * T  # 96
K1 = 2 * T  # 48 stream rows: one-hot hour + mask
MASK_NEG = -1.0e9

# packed bf16 constant tensor [HT+1, CWB]: columns
C_VP = 0                    # vproj_ext [97, 256]
C_SEG = C_VP + D            # seg [97, 4]
C_TAB = C_SEG + H           # table (valid rows :K1) [48, 97]
C_SEGT = C_TAB + (HT + 1)   # segT (valid rows :H) [4, 96]
CWB = C_SEGT + HT


def _host_tables(timeslot_embedded, user, hour, hour_mask, user_pref,
                 Wq, bq, Wk, bk, Wv, bv, Wu, bu):
    import ml_dtypes
    f32 = np.float32
    bf16 = ml_dtypes.bfloat16
    ts_e = np.asarray(timeslot_embedded, f32)          # [T, D]
    user = np.asarray(user).astype(np.int64)           # [B]
    hour = np.asarray(hour).astype(np.int64)           # [B, S]
    hour_mask = np.asarray(hour_mask)                  # [B, S, T]
    Wq = np.asarray(Wq, f32); bq = np.asarray(bq, f32)
    Wk = np.asarray(Wk, f32); bk = np.asarray(bk, f32)
    Wv = np.asarray(Wv, f32); bv = np.asarray(bv, f32)
    Wu = np.asarray(Wu, f32); bu = np.asarray(bu, f32)

    Wq_u, Wq_t = Wq[:, :, :D], Wq[:, :, D:]
    k_ = np.einsum('td,hkd->htk', ts_e, Wk) + bk[:, None, :]   # [H,T,HD]
    v_ = np.einsum('td,hkd->htk', ts_e, Wv) + bv[:, None, :]
    time_q = np.einsum('td,hkd->thk', ts_e, Wq_t)              # [T,H,HD]
    upref = np.asarray(user_pref, f32)[user]                   # [B,D]
    user_q = np.einsum('bd,hkd->bhk', upref, Wq_u) + bq[None]  # [B,H,HD]
    scale = f32(1.0 / np.sqrt(HD))
    TS = (np.einsum('thk,hsk->ths', time_q, k_) * scale).reshape(T, HT)
    US = (np.einsum('bhk,hsk->bhs', user_q, k_) * scale).reshape(B, HT)
    vproj = np.einsum('htk,dhk->htd', v_, Wu.reshape(D, H, HD)).reshape(HT, D)

    # table [K1, HT+1]: rows 0..23 TS, rows 24..47 mask additive; col 96 = 0
    table = np.zeros((K1, HT + 1), f32)
    table[:T, :HT] = TS
    table[T:, :HT] = np.tile(np.eye(T, dtype=f32), (1, H)) * f32(MASK_NEG)

    seg = np.zeros((HT + 1, H), f32)
    seg[:HT] = np.repeat(np.eye(H, dtype=f32), T, axis=0)
    segT = np.ascontiguousarray(seg[:HT].T)                    # [H, HT]

    cb = np.zeros((HT + 1, CWB), f32)
    cb[:HT, C_VP:C_VP + D] = vproj
    cb[HT, C_VP:C_VP + D] = bu
    cb[:, C_SEG:C_SEG + H] = seg
    cb[:K1, C_TAB:C_TAB + HT + 1] = table
    cb[:H, C_SEGT:C_SEGT + HT] = segT
    const_bf = cb.astype(bf16)

    us_all = np.zeros((HT + 1, B), f32)
    us_all[:HT, :] = US.T
    us_cols = [np.ascontiguousarray(us_all[:, c * BPC:(c + 1) * BPC])
               for c in range(NCORES)]

    # per-core streams [BPC, K1, S] bf16: one-hot(hour) rows + mask^T rows
    eyeT = np.eye(T, dtype=f32)
    streams = []
    for c in range(NCORES):
        hb = hour[c * BPC:(c + 1) * BPC]                       # [BPC, S]
        mb = hour_mask[c * BPC:(c + 1) * BPC]                  # [BPC, S, T]
        st = np.empty((BPC, K1, S), f32)
        st[:, :T, :] = eyeT[hb].transpose(0, 2, 1)
        st[:, T:, :] = mb.astype(f32).transpose(0, 2, 1)
        streams.append(st.astype(bf16))
    return const_bf, us_cols, streams


def _build_program():
    import concourse.bass as bass
    import concourse.mybir as mybir
    from contextlib import ExitStack

    f32 = mybir.dt.float32
    bf16 = mybir.dt.bfloat16
    nc = bass.Bass("TRN2")
    stream_d = nc.declare_dram_parameter("stream", [BPC, K1, S], bf16,
                                         isOutput=False)
    const_d = nc.declare_dram_parameter("const", [HT + 1, CWB], bf16,
                                        isOutput=False)
    us_d = nc.declare_dram_parameter("usb", [HT + 1, BPC], f32, isOutput=False)
    out_d = nc.declare_dram_parameter("out", [BPC, D, S], f32, isOutput=True)

    Exp = mybir.ActivationFunctionType.Exp
    Ln = mybir.ActivationFunctionType.Ln

    # Deep software pipeline: stage lags keep the PE matmul stream free of
    # same-iteration ACT/DVE dependencies (stalls reset the PE clock ramp).
    # PE block j emits: bc_{j-2} | mm2ab_{j-3} | hs_j | A_{j+1}.
    # ACT ticks: exp1_i=3i+1 ln=3i+2 expneg=3i+3.
    # DVE ticks: mul_i=2i+1 ocopy_i=2i+2.  PE ticks recorded at emission.
    with ExitStack() as ctx:
        ec = ctx.enter_context
        const_sb = ec(nc.sbuf_tensor("const_sb", [HT + 1, CWB], bf16))
        us_sb = ec(nc.sbuf_tensor("us_sb", [HT + 1, BPC], f32))
        sts = [ec(nc.sbuf_tensor(f"st{j}", [K1, S], bf16)) for j in range(BPC)]
        ps = [ec(nc.sbuf_tensor(f"p{j}", [HT + 1, S], bf16)) for j in range(5)]
        lnz_sb = ec(nc.sbuf_tensor("lnz_sb", [H, S], f32))
        r_sbs = [ec(nc.sbuf_tensor(f"r_sb{j}", [H, S], bf16)) for j in range(3)]
        ots = [ec(nc.sbuf_tensor(f"ot{j}", [128, 2 * S], f32))
               for j in range(2)]
        ps_s = ec(nc.psum_tensor("ps_s", [HT + 1, S], f32))
        ps_zs = [ec(nc.psum_tensor(f"ps_z{j}", [H, S], f32)) for j in range(2)]
        ps_r = ec(nc.psum_tensor("ps_r", [HT, S], f32))
        ps_os = [ec(nc.psum_tensor(f"ps_o{j}", [128, 2 * S], f32))
                 for j in range(2)]
        c_sem = ec(nc.semaphore("c_sem"))
        u_sem = ec(nc.semaphore("u_sem"))
        st_sems = [ec(nc.semaphore(f"st_sem{j}")) for j in range(BPC)]
        pe_sem = ec(nc.semaphore("pe_sem"))
        act_sem = ec(nc.semaphore("act_sem"))
        dve_sem = ec(nc.semaphore("dve_sem"))
        ot_sems = [ec(nc.semaphore(f"ot_sem{j}")) for j in range(2)]
        block = ec(nc.Block())

        vproj = const_sb[:, C_VP:C_VP + D]
        seg = const_sb[:, C_SEG:C_SEG + H]
        table = const_sb[:K1, C_TAB:C_TAB + HT + 1]
        segT = const_sb[:H, C_SEGT:C_SEGT + HT]

        pe_tick = {}
        pe_cnt = [0]

        @block.tensor
        def _(tensor):
            def mm(key, out, lhsT, rhs):
                tensor.matmul(out, lhsT, rhs,
                              start=True, stop=True).then_inc(pe_sem, 1)
                pe_cnt[0] += 1
                pe_tick[key] = pe_cnt[0]

            tensor.wait_ge(c_sem, 16)
            tensor.wait_ge(st_sems[0], 16)
            mm(('A', 0), ps_s[:], table, sts[0][:])
            for j in range(BPC + 3):
                if 0 <= j - 2 < BPC:            # bc_{j-2}
                    i = j - 2
                    tensor.wait_ge(act_sem, 3 * i + 3)   # expneg_i done
                    if i >= 1:
                        tensor.wait_ge(dve_sem, 2 * (i - 1) + 1)  # mul_{i-1}
                    mm(('bc', i), ps_r[:], segT, r_sbs[i % 3][:])
                if 0 <= j - 3 < BPC:            # mm2_{j-3}
                    i = j - 3
                    tensor.wait_ge(dve_sem, 2 * i + 1)   # mul_i done
                    mm(('m2a', i), ps_os[i % 2][:, 0:S],
                       vproj[:, 0:128], ps[i % 5][:])
                    mm(('m2b', i), ps_os[i % 2][:, S:2 * S],
                       vproj[:, 128:256], ps[i % 5][:])
                if j < BPC:                     # hs_j
                    tensor.wait_ge(act_sem, 3 * j + 1)   # exp1_j done
                    mm(('hs', j), ps_zs[j % 2][:], seg, ps[j % 5][:])
                if j + 1 < BPC:                 # A_{j+1}
                    i = j + 1
                    tensor.wait_ge(st_sems[i], 16)
                    tensor.wait_ge(act_sem, 3 * (i - 1) + 1)      # exp1_{i-1}
                    mm(('A', i), ps_s[:], table, sts[i][:])

        @block.scalar
        def _(scalar):
            scalar.wait_ge(u_sem, 16)
            for i in range(BPC):
                scalar.wait_ge(pe_sem, pe_tick[('A', i)])
                scalar.activation(ps[i % 5][:], ps_s[:], Exp,
                                  bias=us_sb[:, i:i + 1],
                                  scale=1.0).then_inc(act_sem, 1)
                scalar.wait_ge(pe_sem, pe_tick[('hs', i)])
                scalar.activation(lnz_sb[:],
                                  ps_zs[i % 2][:], Ln).then_inc(act_sem, 1)
                scalar.activation(r_sbs[i % 3][:], lnz_sb[:], Exp,
                                  scale=-1.0).then_inc(act_sem, 1)

        @block.vector
        def _(vector):
            for i in range(BPC):
                vector.wait_ge(pe_sem, pe_tick[('bc', i)])
                vector.tensor_mul(ps[i % 5][:HT, :], ps[i % 5][:HT, :],
                                  ps_r[:]).then_inc(dve_sem, 1)
                vector.wait_ge(pe_sem, pe_tick[('m2b', i)])
                if i >= 2:
                    vector.wait_ge(ot_sems[i % 2], 16 * (i // 2))
                vector.tensor_copy(ots[i % 2][:],
                                   ps_os[i % 2][:]).then_inc(dve_sem, 1)

        @block.sync
        def _(sync):
            sync.dma_start(sts[0][:], stream_d[0]).then_inc(st_sems[0], 16)
            sync.dma_start(const_sb[:], const_d[:]).then_inc(c_sem, 16)
            for i in range(1, BPC):
                sync.dma_start(sts[i][:], stream_d[i]).then_inc(st_sems[i], 16)
            for i in range(BPC):
                sync.wait_ge(dve_sem, 2 * i + 2)      # ocopy_i done
                dest = out_d[i, :, :].rearrange("(h p) s -> p h s", h=2)
                src = ots[i % 2][:, :].rearrange("p (h s) -> p h s", h=2)
                sync.dma_start(dest, src).then_inc(ot_sems[i % 2], 16)
            for bb in range(2):
                cnt = len([i for i in range(BPC) if i % 2 == bb])
                sync.wait_ge(ot_sems[bb], 16 * cnt)

        @block.gpsimd
        def _(gpsimd):
            gpsimd.dma_start(us_sb[:], us_d[:]).then_inc(u_sem, 16)
    return nc


def _run(inputs, trace=False):
    import sys
    if "/opt/trn_rl_repo" not in sys.path:
        sys.path.insert(0, "/opt/trn_rl_repo")
    from concourse.bass_utils import run_bass_kernel_spmd

    const_bf, us_cols, streams = _host_tables(**inputs)
    nc = _build_program()
    in_maps = [
        {"stream": streams[c], "const": const_bf, "usb": us_cols[c]}
        for c in range(NCORES)
    ]
    res = run_bass_kernel_spmd(nc, in_maps, core_ids=list(range(NCORES)),
                               trace=trace)
    out_full = np.empty((B, S, D), np.float32)
    for c in range(NCORES):
        oc = res.results[c]["out"]  # [BPC, D, S]
        out_full[c * BPC:(c + 1) * BPC] = oc.transpose(0, 2, 1)
    return out_full, res


def kernel(**inputs):
    trace = bool(int(os.environ.get("BASS_KERNEL_TRACE", "0")))
    out, _ = _run(inputs, trace=trace)
    return out


def kernel_profiled(**inputs):
    out, res = _run(inputs, trace=True)
    return out, res


# revision 11
# speedup vs baseline: 3.4384x; 1.1650x over previous
"""Trainium2 Bass kernel for nn_ArrivalTime (sparse attention over 24 timeslots).

Math refactoring (exact, up to fp reassociation):
  query = [user_pref[user], timeslot[hour]] has only 64 distinct user rows and
  24 distinct time rows, so
    scores[n,h,t] = US[b(n), h, t] + TS[hour[n], h, t]
  with tiny host-precomputed tables
    US = (user_q @ k^T) * scale   [64, H*T]   (user_q folds bq)
    TS = (time_q @ k^T) * scale   [24, H*T]
  Masking adds -1e9 where hour_mask==1.  Softmax per head over t (24).
  Output: out[n,:] = attn[n,:] @ vproj + bu, vproj[(h,t),d] = v[h,t,:]@Wu[d,h*HD:]^T.

Device pipeline (per core, transposed layout: tokens on the free dim), one
iteration per batch row (512 tokens):
  PE : ps_s = table^T @ stream      (one-hot hour + mask rows, bf16)
  ACT: p = exp(ps_s + US_b bias)    (row 96 = exp(0) = 1 -> carries bu)
  PE : ps_z = seg^T @ p             (per-head sums)
  ACT: lnz = ln(ps_z); r = exp(-lnz)     (1/Z without the slow DVE reciprocal)
  PE : ps_r = segT^T @ r            (broadcast 1/Z over each head's 24 rows)
  DVE: p[:96] *= ps_r               (normalize)
  PE : ps_o{h} = vproj[:,h]^T @ p   (two 128-col halves; vproj row 96 = bu)
  DVE: copy psum -> sbuf; SYNC triggers the output DMAs.

Sharding: data-parallel over batch, 8 batch rows (= 8 x 512 tokens) per core.
Raw bass (no Tile): this toolchain's walrus allows at most one attached sem
wait per instruction, so all waits are standalone wait_ge ops with manually
counted thresholds.
"""

import os
import numpy as np

B, S, D, H, HD, T = 64, 512, 256, 4, 64, 24
NCORES = 8
BPC = B // NCORES  # batch rows per core
HT = H * T  # 96
K1 = 2 * T  # 48 stream rows: one-hot hour + mask
MASK_NEG = -1.0e9

# packed bf16 constant tensor [HT+1, CWB]: columns
C_VP = 0                    # vproj_ext [97, 256]
C_SEG2 = C_VP + D           # seg2 [97, 96]: 1 where head(k)==head(m)
C_TAB = C_SEG2 + HT         # table (valid rows :K1) [48, 97]
CWB = C_TAB + (HT + 1)


def _host_tables(timeslot_embedded, user, hour, hour_mask, user_pref,
                 Wq, bq, Wk, bk, Wv, bv, Wu, bu):
    import ml_dtypes
    f32 = np.float32
    bf16 = ml_dtypes.bfloat16
    ts_e = np.asarray(timeslot_embedded, f32)          # [T, D]
    user = np.asarray(user).astype(np.int64)           # [B]
    hour = np.asarray(hour).astype(np.int64)           # [B, S]
    hour_mask = np.asarray(hour_mask)                  # [B, S, T]
    Wq = np.asarray(Wq, f32); bq = np.asarray(bq, f32)
    Wk = np.asarray(Wk, f32); bk = np.asarray(bk, f32)
    Wv = np.asarray(Wv, f32); bv = np.asarray(bv, f32)
    Wu = np.asarray(Wu, f32); bu = np.asarray(bu, f32)

    Wq_u, Wq_t = Wq[:, :, :D], Wq[:, :, D:]
    k_ = np.einsum('td,hkd->htk', ts_e, Wk) + bk[:, None, :]   # [H,T,HD]
    v_ = np.einsum('td,hkd->htk', ts_e, Wv) + bv[:, None, :]
    time_q = np.einsum('td,hkd->thk', ts_e, Wq_t)              # [T,H,HD]
    upref = np.asarray(user_pref, f32)[user]                   # [B,D]
    user_q = np.einsum('bd,hkd->bhk', upref, Wq_u) + bq[None]  # [B,H,HD]
    scale = f32(1.0 / np.sqrt(HD))
    TS = (np.einsum('thk,hsk->ths', time_q, k_) * scale).reshape(T, HT)
    US = (np.einsum('bhk,hsk->bhs', user_q, k_) * scale).reshape(B, HT)
    vproj = np.einsum('htk,dhk->htd', v_, Wu.reshape(D, H, HD)).reshape(HT, D)

    # table [K1, HT+1]: rows 0..23 TS, rows 24..47 mask additive; col 96 = 0
    table = np.zeros((K1, HT + 1), f32)
    table[:T, :HT] = TS
    table[T:, :HT] = np.tile(np.eye(T, dtype=f32), (1, H)) * f32(MASK_NEG)

    seg2 = np.kron(np.eye(H, dtype=f32), np.ones((T, T), f32))  # [HT, HT]

    cb = np.zeros((HT + 1, CWB), f32)
    cb[:HT, C_VP:C_VP + D] = vproj
    cb[HT, C_VP:C_VP + D] = bu
    cb[:HT, C_SEG2:C_SEG2 + HT] = seg2
    cb[:K1, C_TAB:C_TAB + HT + 1] = table
    const_bf = cb.astype(bf16)

    us_all = np.zeros((HT + 1, B), f32)
    us_all[:HT, :] = US.T
    us_cols = [np.ascontiguousarray(us_all[:, c * BPC:(c + 1) * BPC])
               for c in range(NCORES)]

    # per-core streams [BPC, K1, S] bf16: one-hot(hour) rows + mask^T rows
    eyeT = np.eye(T, dtype=f32)
    streams = []
    for c in range(NCORES):
        hb = hour[c * BPC:(c + 1) * BPC]                       # [BPC, S]
        mb = hour_mask[c * BPC:(c + 1) * BPC]                  # [BPC, S, T]
        st = np.empty((BPC, K1, S), f32)
        st[:, :T, :] = eyeT[hb].transpose(0, 2, 1)
        st[:, T:, :] = mb.astype(f32).transpose(0, 2, 1)
        streams.append(st.astype(bf16))
    return const_bf, us_cols, streams


def _build_program():
    import concourse.bass as bass
    import concourse.mybir as mybir
    from contextlib import ExitStack

    f32 = mybir.dt.float32
    bf16 = mybir.dt.bfloat16
    nc = bass.Bass("TRN2")
    stream_d = nc.declare_dram_parameter("stream", [BPC, K1, S], bf16,
                                         isOutput=False)
    const_d = nc.declare_dram_parameter("const", [HT + 1, CWB], bf16,
                                        isOutput=False)
    us_d = nc.declare_dram_parameter("usb", [HT + 1, BPC], f32, isOutput=False)
    out_d = nc.declare_dram_parameter("out", [BPC, D, S], f32, isOutput=True)

    Exp = mybir.ActivationFunctionType.Exp
    Ln = mybir.ActivationFunctionType.Ln

    # Deep software pipeline; 4 matmuls per iteration:
    #   A (scores), hs2 (head-sum REPLICATED via seg2 -> [96,S]), mm2 x2.
    # ln/expneg then produce 1/Z already broadcast ([96,S] costs the same as
    # [4,S] on ACT: free-dim bound), and the DVE mul runs bf16-SBUF at 2x.
    # PE block j emits: mm2ab_{j-2} | hs2_j | A_{j+1}.
    # ACT ticks: exp1_i=3i+1 ln=3i+2 expneg=3i+3.
    # DVE ticks: mul_i=2i+1 ocopy_i=2i+2.  PE ticks recorded at emission.
    with ExitStack() as ctx:
        ec = ctx.enter_context
        const_sb = ec(nc.sbuf_tensor("const_sb", [HT + 1, CWB], bf16))
        us_sb = ec(nc.sbuf_tensor("us_sb", [HT + 1, BPC], f32))
        sts = [ec(nc.sbuf_tensor(f"st{j}", [K1, S], bf16)) for j in range(BPC)]
        ps = [ec(nc.sbuf_tensor(f"p{j}", [HT + 1, S], bf16)) for j in range(5)]
        lnz_sb = ec(nc.sbuf_tensor("lnz_sb", [HT, S], f32))
        r_sbs = [ec(nc.sbuf_tensor(f"r_sb{j}", [HT, S], bf16)) for j in range(3)]
        ots = [ec(nc.sbuf_tensor(f"ot{j}", [128, 2 * S], f32))
               for j in range(2)]
        ps_ss = [ec(nc.psum_tensor(f"ps_s{j}", [HT + 1, S], f32))
                 for j in range(2)]
        ps_zs = [ec(nc.psum_tensor(f"ps_z{j}", [HT, S], f32)) for j in range(2)]
        ps_os = [ec(nc.psum_tensor(f"ps_o{j}", [128, 2 * S], f32))
                 for j in range(2)]
        c_sem = ec(nc.semaphore("c_sem"))
        u_sem = ec(nc.semaphore("u_sem"))
        st_sems = [ec(nc.semaphore(f"st_sem{j}")) for j in range(BPC)]
        pe_sem = ec(nc.semaphore("pe_sem"))
        act_sem = ec(nc.semaphore("act_sem"))
        dve_sem = ec(nc.semaphore("dve_sem"))
        ot_sems = [ec(nc.semaphore(f"ot_sem{j}")) for j in range(2)]
        block = ec(nc.Block())

        vproj = const_sb[:, C_VP:C_VP + D]
        seg2 = const_sb[:, C_SEG2:C_SEG2 + HT]
        table = const_sb[:K1, C_TAB:C_TAB + HT + 1]

        pe_tick = {}
        pe_cnt = [0]

        @block.tensor
        def _(tensor):
            def mm(key, out, lhsT, rhs):
                tensor.matmul(out, lhsT, rhs,
                              start=True, stop=True).then_inc(pe_sem, 1)
                pe_cnt[0] += 1
                pe_tick[key] = pe_cnt[0]

            tensor.wait_ge(c_sem, 16)
            tensor.wait_ge(st_sems[0], 16)
            mm(('A', 0), ps_ss[0][:], table, sts[0][:])
            for j in range(BPC + 2):
                if 0 <= j - 2 < BPC:            # mm2_{j-2}
                    i = j - 2
                    tensor.wait_ge(dve_sem, 2 * i + 1)   # mul_i done
                    mm(('m2a', i), ps_os[i % 2][:, 0:S],
                       vproj[:, 0:128], ps[i % 5][:])
                    mm(('m2b', i), ps_os[i % 2][:, S:2 * S],
                       vproj[:, 128:256], ps[i % 5][:])
                if j < BPC:                     # hs2_j
                    tensor.wait_ge(act_sem, 3 * j + 1)   # exp1_j done
                    mm(('hs', j), ps_zs[j % 2][:], seg2, ps[j % 5][:])
                if j + 1 < BPC:                 # A_{j+1}
                    i = j + 1
                    tensor.wait_ge(st_sems[i], 16)
                    mm(('A', i), ps_ss[i % 2][:], table, sts[i][:])

        @block.scalar
        def _(scalar):
            scalar.wait_ge(u_sem, 16)
            for i in range(BPC):
                scalar.wait_ge(pe_sem, pe_tick[('A', i)])
                scalar.activation(ps[i % 5][:], ps_ss[i % 2][:], Exp,
                                  bias=us_sb[:, i:i + 1],
                                  scale=1.0).then_inc(act_sem, 1)
                scalar.wait_ge(pe_sem, pe_tick[('hs', i)])
                scalar.activation(lnz_sb[:],
                                  ps_zs[i % 2][:], Ln).then_inc(act_sem, 1)
                scalar.activation(r_sbs[i % 3][:], lnz_sb[:], Exp,
                                  scale=-1.0).then_inc(act_sem, 1)

        @block.vector
        def _(vector):
            for i in range(BPC):
                vector.wait_ge(pe_sem, pe_tick[('hs', i)])
                vector.wait_ge(act_sem, 3 * i + 3)    # expneg_i done
                vector.tensor_mul(ps[i % 5][:HT, :], ps[i % 5][:HT, :],
                                  r_sbs[i % 3][:]).then_inc(dve_sem, 1)
                vector.wait_ge(pe_sem, pe_tick[('m2b', i)])
                if i >= 2:
                    vector.wait_ge(ot_sems[i % 2], 16 * (i // 2))
                vector.tensor_copy(ots[i % 2][:],
                                   ps_os[i % 2][:]).then_inc(dve_sem, 1)

        @block.sync
        def _(sync):
            sync.dma_start(sts[0][:], stream_d[0]).then_inc(st_sems[0], 16)
            sync.dma_start(const_sb[:], const_d[:]).then_inc(c_sem, 16)
            for i in range(1, BPC):
                sync.dma_start(sts[i][:], stream_d[i]).then_inc(st_sems[i], 16)
            for i in range(BPC):
                sync.wait_ge(dve_sem, 2 * i + 2)      # ocopy_i done
                dest = out_d[i, :, :].rearrange("(h p) s -> p h s", h=2)
                src = ots[i % 2][:, :].rearrange("p (h s) -> p h s", h=2)
                sync.dma_start(dest, src).then_inc(ot_sems[i % 2], 16)
            for bb in range(2):
                cnt = len([i for i in range(BPC) if i % 2 == bb])
                sync.wait_ge(ot_sems[bb], 16 * cnt)

        @block.gpsimd
        def _(gpsimd):
            gpsimd.dma_start(us_sb[:], us_d[:]).then_inc(u_sem, 16)
    return nc


def _run(inputs, trace=False):
    import sys
    if "/opt/trn_rl_repo" not in sys.path:
        sys.path.insert(0, "/opt/trn_rl_repo")
    from concourse.bass_utils import run_bass_kernel_spmd

    const_bf, us_cols, streams = _host_tables(**inputs)
    nc = _build_program()
    in_maps = [
        {"stream": streams[c], "const": const_bf, "usb": us_cols[c]}
        for c in range(NCORES)
    ]
    res = run_bass_kernel_spmd(nc, in_maps, core_ids=list(range(NCORES)),
                               trace=trace)
    out_full = np.empty((B, S, D), np.float32)
    for c in range(NCORES):
        oc = res.results[c]["out"]  # [BPC, D, S]
        out_full[c * BPC:(c + 1) * BPC] = oc.transpose(0, 2, 1)
    return out_full, res


def kernel(**inputs):
    trace = bool(int(os.environ.get("BASS_KERNEL_TRACE", "0")))
    out, _ = _run(inputs, trace=trace)
    return out


def kernel_profiled(**inputs):
    out, res = _run(inputs, trace=True)
    return out, res
